# revision 4
# baseline (speedup 1.0000x reference)
"""Trainium2 Bass kernel for the 3x3 abs-diff stencil module:

    out = x + alpha * sum_{di,dj in 3x3} |x - shift_{di,dj}(zero_pad(x))|

x: (8, 64, 256, 256) f32, alpha: (1, 64, 1, 1) f32.

Strategy (pure data parallel, no collectives; core i <- batch i):

  - Host pre-casts x to bf16 and zero-pads each shard to (64, 258, 260).
  - SBUF layout: partition p = (s, c), s = H-half, c = channel; free dim =
    (rows, cols). All stencil shifts are free-dim AP offsets.
  - Per 16-row job, 4 signed diff fields are computed on DVE (bf16 2x):
      dE[t,u]  = xp[t+1,u]   - xp[t+1,u+1]   (horizontal)
      dS[t,w]  = xp[t,w+2]   - xp[t+1,w+2]   (vertical)
      dSE[t,u] = xp[t,u+1]   - xp[t+1,u+2]   (diagonal \\)
      dSW[t,u] = xp[t,u+2]   - xp[t+1,u+1]   (diagonal /)
    abs is split across engines: dE,dS via DVE int32-AND (2 int32/cyc),
    dSE,dSW via ACT Abs.
  - The 8-neighbor sum runs entirely on the PE: for each PSUM bank
    (2 output rows x 256 cols) 4 matmuls accumulate the 8 shifted terms,
    each matmul folding a +/- direction PAIR of the same field through a
    4-axis moving AP whose output AP repeats the bank slots (stride-0 pair
    axis -> PSUM accumulates both passes).  The stationary matrix is
    diag(alpha) in bf16 for every matmul in the kernel, so PSUM ends up
    holding alpha * S directly and the drain is a plain ACT copy to bf16.
  - DVE then adds x into the drained tile (out = x + alpha*S) and the
    result is stored as bf16 (host casts back to f32).
  - Emission is software-pipelined as in the v1 kernel: ACT abs of job j
    precede the PSUM drains of job j-1; final add + store of job j-2 are
    emitted inside iteration j.  Loads ride HWDGE on the idle SP queue,
    stores ride SWDGE on the otherwise idle Pool queue so neither blocks
    the other at a queue head.

Per-core engine budget (theory): DVE ~110us, PE ~109us, ACT ~93us.
"""

import sys

import numpy as np

try:
    import concourse  # noqa: F401
except ImportError:
    sys.path.insert(0, "/opt/trn_rl_repo")

from contextlib import ExitStack

import concourse.bacc as bacc
import concourse.bass as bass
import concourse.mybir as mybir
import concourse.tile as tile
from concourse.bass_utils import run_bass_kernel_spmd

F32 = mybir.dt.float32
BF16 = mybir.dt.bfloat16

C = 64
N_CORES = 8


def build_graph(H=256, W=256, J=16):
    """Build the per-core Bass graph (identical on all 8 cores).

    Input DRAM tensor per core: (C, H+2, W+4) bf16 host-padded;
    output (C, H, W) bf16; adiag (128, 128) bf16 = diag(alpha).
    """
    HP, WP = H + 2, W + 4
    HH = H // 2          # rows per half
    NJ = HH // J         # jobs; each job covers both halves via partitions
    assert HH % J == 0 and J % 4 == 0

    nc = bacc.Bacc("TRN2", target_bir_lowering=False, debug=False,
                   num_devices=N_CORES)
    x_d = nc.dram_tensor("x", [C, HP, WP], BF16, kind="ExternalInput")
    a_d = nc.dram_tensor("adiag", [128, 128], BF16, kind="ExternalInput")
    o_d = nc.dram_tensor("out", [C, H, W], BF16, kind="ExternalOutput")

    sub = mybir.AluOpType.subtract
    Copy = mybir.ActivationFunctionType.Copy
    Abs = mybir.ActivationFunctionType.Abs

    with tile.TileContext(nc) as tc, ExitStack() as ctx:
        const_pool = ctx.enter_context(tc.tile_pool(name="const", bufs=1))
        xp_pool = ctx.enter_context(tc.tile_pool(name="xp", bufs=6))
        d_pool = ctx.enter_context(tc.tile_pool(name="d", bufs=2))
        o_pool = ctx.enter_context(tc.tile_pool(name="o", bufs=3))
        ps_pool = ctx.enter_context(tc.tile_pool(name="ps", bufs=4, space="PSUM"))

        adiag_t = const_pool.tile([128, 128], BF16, name="adiag_t")
        nc.sync.dma_start(out=adiag_t[:], in_=a_d.ap())

        def drain_stage(ps_list, o_t):
            # ACT: PSUM (= alpha*S, f32) -> bf16 o_t rows; must complete
            # before the next job's matmuls reuse the banks
            for r, ps in enumerate(ps_list):
                nc.scalar.activation(o_t[:, 4 * r:4 * r + 4, :], ps[:], Copy)

        def final_stage(j, o_t, xp):
            # DVE: out = alpha*S + x ; then SWDGE store on the Pool queue
            nc.vector.tensor_add(o_t[:], o_t[:], xp[:, 1:J + 1, 2:W + 2])
            dst = bass.AP(o_d, J * j * W,
                          [[HH * W, 2], [H * W, C], [W, J], [1, W]])
            nc.gpsimd.dma_start(out=dst, in_=o_t[:])

        def term_matmul(ps, g, d_t, elem_off, row_stride, start, stop):
            # one matmul accumulating one shifted |diff| term (2 rows x 256)
            # into PSUM bank slice g, stationary = diag(alpha)
            mv = bass.AP(d_t.tensor, d_t.offset + elem_off,
                         [list(d_t.ap[0]), [row_stride, 2], [1, W]])
            nc.tensor.matmul(ps[:, 512 * g:512 * g + 512], adiag_t[:], mv,
                             start=start, stop=stop)

        pending = None    # (j, ps_list, o_t, xp) of the previous job
        fin_pending = []  # [(j, o_t, xp)] awaiting final+store
        for j in range(NJ):
            r0 = J * j  # padded-row start of this job within each half

            # ---- load (HWDGE on SP; one contiguous run per partition)
            xp = xp_pool.tile([128, J + 2, WP], BF16, name="xp", tag="xp")
            src = bass.AP(x_d, r0 * WP,
                          [[HH * WP, 2], [HP * WP, C], [WP, J + 2], [1, WP]])
            nc.sync.dma_start(out=xp[:], in_=src)

            # ---- 4 signed diff fields on DVE (bf16 2x streams)
            dE = d_pool.tile([128, J, WP - 2], BF16, name="dE", tag="dE")
            dS = d_pool.tile([128, J + 1, W], BF16, name="dS", tag="dS")
            dSE = d_pool.tile([128, J + 1, WP - 2], BF16, name="dSE", tag="dSE")
            dSW = d_pool.tile([128, J + 1, WP - 2], BF16, name="dSW", tag="dSW")
            WD = WP - 2  # 258: diff-tile width

            nc.vector.tensor_tensor(dE[:], xp[:, 1:J + 1, 0:WD],
                                    xp[:, 1:J + 1, 1:WD + 1], sub)
            nc.vector.tensor_tensor(dS[:], xp[:, 0:J + 1, 2:W + 2],
                                    xp[:, 1:J + 2, 2:W + 2], sub)
            nc.vector.tensor_tensor(dSE[:], xp[:, 0:J + 1, 1:WD + 1],
                                    xp[:, 1:J + 2, 2:WD + 2], sub)
            nc.vector.tensor_tensor(dSW[:], xp[:, 0:J + 1, 2:WD + 2],
                                    xp[:, 1:J + 2, 1:WD + 1], sub)

            # ---- abs: dE,dS in place on DVE (int32 AND clears the packed
            # bf16 sign bits at 2 int32/cycle)
            for dt_ in (dE, dS):
                flat = dt_[:, :, :].rearrange("p r w -> p (r w)")
                flat_i = flat.bitcast(mybir.dt.int32)
                nc.vector.tensor_scalar(flat_i, flat_i, 0x7FFF7FFF, None,
                                        mybir.AluOpType.bitwise_and)
            # dSE,dSW on ACT (before the drains of j-1 in the ACT queue so
            # the queue head never waits on the previous job's matmuls)
            nc.scalar.activation(dSE[:], dSE[:], Abs)
            nc.scalar.activation(dSW[:], dSW[:], Abs)

            # ---- pipelined late stages
            if pending is not None:
                drain_stage(pending[1], pending[2])
                fin_pending.append((pending[0], pending[2], pending[3]))
            if len(fin_pending) >= 2:
                final_stage(*fin_pending.pop(0))

            # ---- 8-term accumulate on PE: per bank (2 out rows) 4 pair
            # matmuls with diag(alpha) stationary
            o_t = o_pool.tile([128, J, W], BF16, name="o_t", tag="o")
            ps_list = []
            for r in range(J // 4):
                ps = ps_pool.tile([128, 4 * W], F32, name="ps", tag="ps")
                ps_list.append(ps)
                for g in range(2):      # one PSUM bank per g (2 rows x 256)
                    rr = 4 * r + 2 * g  # first pixel row (q-coord rr+1)
                    # W@ dE[rr..rr+1, 1:257]   E@ dE[rr..rr+1, 2:258]
                    # N@ dS[rr..rr+1, 0:256]   S@ dS[rr+1..rr+2, 0:256]
                    # NW@dSE[rr..rr+1, 0:256]  SE@dSE[rr+1..rr+2, 1:257]
                    # NE@dSW[rr..rr+1, 1:257]  SW@dSW[rr+1..rr+2, 0:256]
                    terms = (
                        (dE, rr * WD + 1, WD),
                        (dE, rr * WD + 2, WD),
                        (dS, rr * W, W),
                        (dS, (rr + 1) * W, W),
                        (dSE, rr * WD, WD),
                        (dSE, (rr + 1) * WD + 1, WD),
                        (dSW, rr * WD + 1, WD),
                        (dSW, (rr + 1) * WD, WD),
                    )
                    for t, (d_t, off, rstr) in enumerate(terms):
                        term_matmul(ps, g, d_t, off, rstr,
                                    start=(t == 0), stop=(t == len(terms) - 1))
            pending = (j, ps_list, o_t, xp)

        drain_stage(pending[1], pending[2])
        fin_pending.append((pending[0], pending[2], pending[3]))
        for fp in fin_pending:
            final_stage(*fp)

    nc.compile()
    return nc


def _prep_inputs(x, alpha, H=256, W=256):
    """Shard batch across cores, cast to bf16 and zero-pad on host."""
    import ml_dtypes
    x = np.asarray(x, dtype=np.float32)
    alpha = np.asarray(alpha, dtype=np.float32).reshape(C)
    B = x.shape[0]
    HP, WP = H + 2, W + 4
    adiag = np.zeros((128, 128), dtype=np.float32)
    idx = np.arange(128)
    adiag[idx, idx] = alpha[idx % C]
    adiag = adiag.astype(ml_dtypes.bfloat16)
    in_maps = []
    for i in range(B):
        xs = np.zeros((C, HP, WP), dtype=ml_dtypes.bfloat16)
        xs[:, 1:H + 1, 2:W + 2] = x[i].astype(ml_dtypes.bfloat16)
        in_maps.append({"x": xs, "adiag": adiag})
    return in_maps


_GRAPH_CACHE = {}


def _get_graph(H=256, W=256, J=16):
    key = (H, W, J)
    if key not in _GRAPH_CACHE:
        _GRAPH_CACHE[key] = build_graph(H, W, J)
    return _GRAPH_CACHE[key]


def kernel(x, alpha, _profile=False):
    x = np.asarray(x, dtype=np.float32)
    alpha = np.asarray(alpha, dtype=np.float32)
    B, c, H, W = x.shape
    assert c == C and B == N_CORES, (B, c, H, W)
    nc = _get_graph(H, W)
    in_maps = _prep_inputs(x, alpha, H, W)
    res = run_bass_kernel_spmd(nc, in_maps, core_ids=list(range(N_CORES)),
                               trace=_profile)
    out = np.stack([res.results[i]["out"].astype(np.float32)
                    for i in range(N_CORES)], axis=0)
    if _profile:
        return out, res
    return out


def kernel_profiled(x, alpha):
    out, res = kernel(x, alpha, _profile=True)
    return out, res.exec_time_ns


# revision 5
# speedup vs baseline: 1.0520x; 1.0520x over previous
"""Trainium2 Bass kernel for the 3x3 abs-diff stencil module:

    out = x + alpha * sum_{di,dj in 3x3} |x - shift_{di,dj}(zero_pad(x))|

x: (8, 64, 256, 256) f32, alpha: (1, 64, 1, 1) f32.

Strategy (pure data parallel, no collectives; core i <- batch i):

  - Host pre-casts x to bf16 and zero-pads each shard to (64, 258, 260).
  - SBUF layout: partition p = (s, c), s = H-half, c = channel; free dim =
    (rows, cols). All stencil shifts are free-dim AP offsets.
  - Per 16-row job, 4 signed diff fields are computed on DVE (bf16 2x):
      dE[t,u]  = xp[t+1,u]   - xp[t+1,u+1]   (horizontal)
      dS[t,w]  = xp[t,w+2]   - xp[t+1,w+2]   (vertical)
      dSE[t,u] = xp[t,u+1]   - xp[t+1,u+2]   (diagonal \\)
      dSW[t,u] = xp[t,u+2]   - xp[t+1,u+1]   (diagonal /)
    abs is split across engines: dE,dS via DVE int32-AND (2 int32/cyc),
    dSE,dSW via ACT Abs.
  - The 8-neighbor sum runs entirely on the PE: for each PSUM bank
    (2 output rows x 256 cols) 4 matmuls accumulate the 8 shifted terms,
    each matmul folding a +/- direction PAIR of the same field through a
    4-axis moving AP whose output AP repeats the bank slots (stride-0 pair
    axis -> PSUM accumulates both passes).  The stationary matrix is
    diag(alpha) in bf16 for every matmul in the kernel, so PSUM ends up
    holding alpha * S directly and the drain is a plain ACT copy to bf16.
  - DVE then adds x into the drained tile (out = x + alpha*S) and the
    result is stored as bf16 (host casts back to f32).
  - Emission is software-pipelined as in the v1 kernel: ACT abs of job j
    precede the PSUM drains of job j-1; final add + store of job j-2 are
    emitted inside iteration j.  Loads ride HWDGE on the idle SP queue,
    stores ride SWDGE on the otherwise idle Pool queue so neither blocks
    the other at a queue head.

Per-core engine budget (theory): DVE ~110us, PE ~109us, ACT ~93us.
"""

import sys

import numpy as np

try:
    import concourse  # noqa: F401
except ImportError:
    sys.path.insert(0, "/opt/trn_rl_repo")

from contextlib import ExitStack

import concourse.bacc as bacc
import concourse.bass as bass
import concourse.mybir as mybir
import concourse.tile as tile
from concourse.bass_utils import run_bass_kernel_spmd

F32 = mybir.dt.float32
BF16 = mybir.dt.bfloat16

C = 64
N_CORES = 8


def build_graph(H=256, W=256, J=16):
    """Build the per-core Bass graph (identical on all 8 cores).

    Input DRAM tensor per core: (C, H+2, W+4) bf16 host-padded;
    output (C, H, W) bf16; adiag (128, 128) bf16 = diag(alpha).
    """
    HP, WP = H + 2, W + 4
    HH = H // 2          # rows per half
    NJ = HH // J         # jobs; each job covers both halves via partitions
    assert HH % J == 0 and J % 4 == 0

    nc = bacc.Bacc("TRN2", target_bir_lowering=False, debug=False,
                   num_devices=N_CORES)
    x_d = nc.dram_tensor("x", [C, HP, WP], BF16, kind="ExternalInput")
    a_d = nc.dram_tensor("adiag", [128, 128], BF16, kind="ExternalInput")
    o_d = nc.dram_tensor("out", [C, H, W], BF16, kind="ExternalOutput")

    sub = mybir.AluOpType.subtract
    Copy = mybir.ActivationFunctionType.Copy
    Abs = mybir.ActivationFunctionType.Abs

    with tile.TileContext(nc) as tc, ExitStack() as ctx:
        const_pool = ctx.enter_context(tc.tile_pool(name="const", bufs=1))
        xp_pool = ctx.enter_context(tc.tile_pool(name="xp", bufs=6))
        d_pool = ctx.enter_context(tc.tile_pool(name="d", bufs=2))
        o_pool = ctx.enter_context(tc.tile_pool(name="o", bufs=3))
        ps_pool = ctx.enter_context(tc.tile_pool(name="ps", bufs=4, space="PSUM"))

        adiag_t = const_pool.tile([128, 128], BF16, name="adiag_t")
        nc.sync.dma_start(out=adiag_t[:], in_=a_d.ap())

        def drain_stage(ps_list, o_t):
            # ACT: PSUM (= alpha*S, f32) -> bf16 o_t rows; must complete
            # before the next job's matmuls reuse the banks
            for r, ps in enumerate(ps_list):
                nc.scalar.activation(o_t[:, 4 * r:4 * r + 4, :], ps[:], Copy)

        def final_stage(j, o_t, xp):
            # DVE: out = alpha*S + x ; then SWDGE store on the Pool queue
            nc.vector.tensor_add(o_t[:], o_t[:], xp[:, 1:J + 1, 2:W + 2])
            dst = bass.AP(o_d, J * j * W,
                          [[HH * W, 2], [H * W, C], [W, J], [1, W]])
            nc.gpsimd.dma_start(out=dst, in_=o_t[:])

        def term_matmul(ps, g, d_t, elem_off, row_stride, start, stop):
            # one matmul accumulating one shifted |diff| term (2 rows x 256)
            # into PSUM bank slice g, stationary = diag(alpha)
            mv = bass.AP(d_t.tensor, d_t.offset + elem_off,
                         [list(d_t.ap[0]), [row_stride, 2], [1, W]])
            nc.tensor.matmul(ps[:, 512 * g:512 * g + 512], adiag_t[:], mv,
                             start=start, stop=stop)

        pending = None    # (j, ps_list, o_t, xp) of the previous job
        fin_pending = []  # [(j, o_t, xp)] awaiting final+store
        for j in range(NJ):
            r0 = J * j  # padded-row start of this job within each half

            # ---- load (SWDGE; one contiguous descriptor per partition)
            xp = xp_pool.tile([128, J + 2, WP], BF16, name="xp", tag="xp")
            src = bass.AP(x_d, r0 * WP,
                          [[HH * WP, 2], [HP * WP, C], [WP, J + 2], [1, WP]])
            nc.gpsimd.dma_start(out=xp[:], in_=src)

            # ---- 4 signed diff fields on DVE (bf16 2x streams)
            dE = d_pool.tile([128, J, WP - 2], BF16, name="dE", tag="dE")
            dS = d_pool.tile([128, J + 1, W], BF16, name="dS", tag="dS")
            dSE = d_pool.tile([128, J + 1, WP - 2], BF16, name="dSE", tag="dSE")
            dSW = d_pool.tile([128, J + 1, WP - 2], BF16, name="dSW", tag="dSW")
            WD = WP - 2  # 258: diff-tile width

            nc.vector.tensor_tensor(dE[:], xp[:, 1:J + 1, 0:WD],
                                    xp[:, 1:J + 1, 1:WD + 1], sub)
            nc.vector.tensor_tensor(dS[:], xp[:, 0:J + 1, 2:W + 2],
                                    xp[:, 1:J + 2, 2:W + 2], sub)
            nc.vector.tensor_tensor(dSE[:], xp[:, 0:J + 1, 1:WD + 1],
                                    xp[:, 1:J + 2, 2:WD + 2], sub)
            nc.vector.tensor_tensor(dSW[:], xp[:, 0:J + 1, 2:WD + 2],
                                    xp[:, 1:J + 2, 1:WD + 1], sub)

            # ---- abs: dE,dS in place on DVE (int32 AND clears the packed
            # bf16 sign bits at 2 int32/cycle)
            for dt_ in (dE, dS):
                flat = dt_[:, :, :].rearrange("p r w -> p (r w)")
                flat_i = flat.bitcast(mybir.dt.int32)
                nc.vector.tensor_scalar(flat_i, flat_i, 0x7FFF7FFF, None,
                                        mybir.AluOpType.bitwise_and)
            # dSE,dSW on ACT (before the drains of j-1 in the ACT queue so
            # the queue head never waits on the previous job's matmuls)
            nc.scalar.activation(dSE[:], dSE[:], Abs)
            nc.scalar.activation(dSW[:], dSW[:], Abs)

            # ---- pipelined late stages
            if pending is not None:
                drain_stage(pending[1], pending[2])
                fin_pending.append((pending[0], pending[2], pending[3]))
            if len(fin_pending) >= 2:
                final_stage(*fin_pending.pop(0))

            # ---- 8-term accumulate on PE: per bank (2 out rows) 4 pair
            # matmuls with diag(alpha) stationary
            o_t = o_pool.tile([128, J, W], BF16, name="o_t", tag="o")
            ps_list = []
            for r in range(J // 4):
                ps = ps_pool.tile([128, 4 * W], F32, name="ps", tag="ps")
                ps_list.append(ps)
                for g in range(2):      # one PSUM bank per g (2 rows x 256)
                    rr = 4 * r + 2 * g  # first pixel row (q-coord rr+1)
                    # W@ dE[rr..rr+1, 1:257]   E@ dE[rr..rr+1, 2:258]
                    # N@ dS[rr..rr+1, 0:256]   S@ dS[rr+1..rr+2, 0:256]
                    # NW@dSE[rr..rr+1, 0:256]  SE@dSE[rr+1..rr+2, 1:257]
                    # NE@dSW[rr..rr+1, 1:257]  SW@dSW[rr+1..rr+2, 0:256]
                    terms = (
                        (dE, rr * WD + 1, WD),
                        (dE, rr * WD + 2, WD),
                        (dS, rr * W, W),
                        (dS, (rr + 1) * W, W),
                        (dSE, rr * WD, WD),
                        (dSE, (rr + 1) * WD + 1, WD),
                        (dSW, rr * WD + 1, WD),
                        (dSW, (rr + 1) * WD, WD),
                    )
                    for t, (d_t, off, rstr) in enumerate(terms):
                        term_matmul(ps, g, d_t, off, rstr,
                                    start=(t == 0), stop=(t == len(terms) - 1))
            pending = (j, ps_list, o_t, xp)

        drain_stage(pending[1], pending[2])
        fin_pending.append((pending[0], pending[2], pending[3]))
        for fp in fin_pending:
            final_stage(*fp)

    nc.compile()
    return nc


def _prep_inputs(x, alpha, H=256, W=256):
    """Shard batch across cores, cast to bf16 and zero-pad on host."""
    import ml_dtypes
    x = np.asarray(x, dtype=np.float32)
    alpha = np.asarray(alpha, dtype=np.float32).reshape(C)
    B = x.shape[0]
    HP, WP = H + 2, W + 4
    adiag = np.zeros((128, 128), dtype=np.float32)
    idx = np.arange(128)
    adiag[idx, idx] = alpha[idx % C]
    adiag = adiag.astype(ml_dtypes.bfloat16)
    in_maps = []
    for i in range(B):
        xs = np.zeros((C, HP, WP), dtype=ml_dtypes.bfloat16)
        xs[:, 1:H + 1, 2:W + 2] = x[i].astype(ml_dtypes.bfloat16)
        in_maps.append({"x": xs, "adiag": adiag})
    return in_maps


_GRAPH_CACHE = {}


def _get_graph(H=256, W=256, J=16):
    key = (H, W, J)
    if key not in _GRAPH_CACHE:
        _GRAPH_CACHE[key] = build_graph(H, W, J)
    return _GRAPH_CACHE[key]


def kernel(x, alpha, _profile=False):
    x = np.asarray(x, dtype=np.float32)
    alpha = np.asarray(alpha, dtype=np.float32)
    B, c, H, W = x.shape
    assert c == C and B == N_CORES, (B, c, H, W)
    nc = _get_graph(H, W)
    in_maps = _prep_inputs(x, alpha, H, W)
    res = run_bass_kernel_spmd(nc, in_maps, core_ids=list(range(N_CORES)),
                               trace=_profile)
    out = np.stack([res.results[i]["out"].astype(np.float32)
                    for i in range(N_CORES)], axis=0)
    if _profile:
        return out, res
    return out


def kernel_profiled(x, alpha):
    out, res = kernel(x, alpha, _profile=True)
    return out, res.exec_time_ns


# revision 8
# speedup vs baseline: 1.0909x; 1.0370x over previous
"""Trainium2 Bass kernel for the 3x3 abs-diff stencil module:

    out = x + alpha * sum_{di,dj in 3x3} |x - shift_{di,dj}(zero_pad(x))|

x: (8, 64, 256, 256) f32, alpha: (1, 64, 1, 1) f32.

Strategy (pure data parallel, no collectives; core i <- batch i):

  - Host pre-casts x to bf16 and zero-pads each shard to (64, 258, 260).
  - SBUF layout: partition p = (s, c), s = H-half, c = channel; free dim =
    (rows, cols). All stencil shifts are free-dim AP offsets.
  - Per 16-row job, 4 signed diff fields are computed on DVE (bf16 2x):
      dE[t,u]  = xp[t+1,u]   - xp[t+1,u+1]   (horizontal)
      dS[t,w]  = xp[t,w+2]   - xp[t+1,w+2]   (vertical)
      dSE[t,u] = xp[t,u+1]   - xp[t+1,u+2]   (diagonal \\)
      dSW[t,u] = xp[t,u+2]   - xp[t+1,u+1]   (diagonal /)
    abs is split across engines: dE,dS via DVE int32-AND (2 int32/cyc),
    dSE,dSW via ACT Abs.
  - The 8-neighbor sum runs entirely on the PE: for each PSUM bank
    (2 output rows x 256 cols) 4 matmuls accumulate the 8 shifted terms,
    each matmul folding a +/- direction PAIR of the same field through a
    4-axis moving AP whose output AP repeats the bank slots (stride-0 pair
    axis -> PSUM accumulates both passes).  The stationary matrix is
    diag(alpha) in bf16 for every matmul in the kernel, so PSUM ends up
    holding alpha * S directly and the drain is a plain ACT copy to bf16.
  - DVE then adds x into the drained tile (out = x + alpha*S) and the
    result is stored as bf16 (host casts back to f32).
  - Emission is software-pipelined as in the v1 kernel: ACT abs of job j
    precede the PSUM drains of job j-1; final add + store of job j-2 are
    emitted inside iteration j.  Loads ride HWDGE on the idle SP queue,
    stores ride SWDGE on the otherwise idle Pool queue so neither blocks
    the other at a queue head.

Per-core engine budget (theory): DVE ~110us, PE ~109us, ACT ~93us.
"""

import sys

import numpy as np

try:
    import concourse  # noqa: F401
except ImportError:
    sys.path.insert(0, "/opt/trn_rl_repo")

from contextlib import ExitStack

import concourse.bacc as bacc
import concourse.bass as bass
import concourse.mybir as mybir
import concourse.tile as tile
from concourse.bass_utils import run_bass_kernel_spmd

F32 = mybir.dt.float32
BF16 = mybir.dt.bfloat16

C = 64
N_CORES = 8


def build_graph(H=256, W=256, J=16):
    """Build the per-core Bass graph (identical on all 8 cores).

    Input DRAM tensor per core: (C, H+2, W+4) bf16 host-padded;
    output (C, H, W) bf16; adiag (128, 128) bf16 = diag(alpha).
    """
    HP, WP = H + 2, W + 4
    HH = H // 2          # rows per half
    NJ = HH // J         # jobs; each job covers both halves via partitions
    assert HH % J == 0 and J % 4 == 0

    nc = bacc.Bacc("TRN2", target_bir_lowering=False, debug=False,
                   num_devices=N_CORES)
    x_d = nc.dram_tensor("x", [C, HP, WP], BF16, kind="ExternalInput")
    a_d = nc.dram_tensor("adiag", [128, 128], BF16, kind="ExternalInput")
    o_d = nc.dram_tensor("out", [C, H, W], BF16, kind="ExternalOutput")

    sub = mybir.AluOpType.subtract
    Copy = mybir.ActivationFunctionType.Copy
    Abs = mybir.ActivationFunctionType.Abs

    with tile.TileContext(nc) as tc, ExitStack() as ctx:
        const_pool = ctx.enter_context(tc.tile_pool(name="const", bufs=1))
        xp_pool = ctx.enter_context(tc.tile_pool(name="xp", bufs=3))
        d_pool = ctx.enter_context(tc.tile_pool(name="d", bufs=2))
        o_pool = ctx.enter_context(tc.tile_pool(name="o", bufs=3))
        ps_pool = ctx.enter_context(tc.tile_pool(name="ps", bufs=4, space="PSUM"))

        adiag_t = const_pool.tile([128, 128], BF16, name="adiag_t")
        nc.sync.dma_start(out=adiag_t[:], in_=a_d.ap())

        def drain_stage(ps_list, o_t):
            # ACT: PSUM (= alpha*S, f32) -> bf16 o_t rows; must complete
            # before the next job's matmuls reuse the banks
            for r, ps in enumerate(ps_list):
                nc.scalar.activation(o_t[:, 4 * r:4 * r + 4, :], ps[:], Copy)

        def final_stage(j, o_t, xp):
            # DVE: out = alpha*S + x ; then SWDGE store on the Pool queue.
            # One flat free axis -> one 8KB descriptor per partition.
            nc.vector.tensor_add(o_t[:], o_t[:], xp[:, 1:J + 1, 2:W + 2])
            dst = bass.AP(o_d, J * j * W,
                          [[HH * W, 2], [H * W, C], [1, J * W]])
            nc.gpsimd.dma_start(out=dst,
                                in_=o_t[:].rearrange("p r w -> p (r w)"))

        def term_matmul(ps, g, d_t, elem_off, row_stride, start, stop):
            # one matmul accumulating one shifted |diff| term (2 rows x 256)
            # into PSUM bank slice g, stationary = diag(alpha)
            mv = bass.AP(d_t.tensor, d_t.offset + elem_off,
                         [list(d_t.ap[0]), [row_stride, 2], [1, W]])
            nc.tensor.matmul(ps[:, 512 * g:512 * g + 512], adiag_t[:], mv,
                             start=start, stop=stop)

        pending = None    # (j, ps_list, o_t, xp) of the previous job
        fin_pending = []  # [(j, o_t, xp)] awaiting final+store
        for j in range(NJ):
            r0 = J * j  # padded-row start of this job within each half

            # ---- load (SWDGE; one contiguous descriptor per partition)
            xp = xp_pool.tile([128, J + 2, WP], BF16, name="xp", tag="xp")
            src = bass.AP(x_d, r0 * WP,
                          [[HH * WP, 2], [HP * WP, C], [1, (J + 2) * WP]])
            nc.gpsimd.dma_start(out=xp[:].rearrange("p r w -> p (r w)"),
                                in_=src)

            # ---- 4 signed diff fields on DVE (bf16 2x streams)
            dE = d_pool.tile([128, J, WP - 2], BF16, name="dE", tag="dE")
            dS = d_pool.tile([128, J + 1, W], BF16, name="dS", tag="dS")
            dSE = d_pool.tile([128, J + 1, WP - 2], BF16, name="dSE", tag="dSE")
            dSW = d_pool.tile([128, J + 1, WP - 2], BF16, name="dSW", tag="dSW")
            WD = WP - 2  # 258: diff-tile width

            nc.vector.tensor_tensor(dE[:], xp[:, 1:J + 1, 0:WD],
                                    xp[:, 1:J + 1, 1:WD + 1], sub)
            nc.vector.tensor_tensor(dS[:], xp[:, 0:J + 1, 2:W + 2],
                                    xp[:, 1:J + 2, 2:W + 2], sub)
            nc.vector.tensor_tensor(dSE[:], xp[:, 0:J + 1, 1:WD + 1],
                                    xp[:, 1:J + 2, 2:WD + 2], sub)
            nc.vector.tensor_tensor(dSW[:], xp[:, 0:J + 1, 2:WD + 2],
                                    xp[:, 1:J + 2, 1:WD + 1], sub)

            # ---- abs: dE,dS in place on DVE (int32 AND clears the packed
            # bf16 sign bits at 2 int32/cycle)
            for dt_ in (dE, dS):
                flat = dt_[:, :, :].rearrange("p r w -> p (r w)")
                flat_i = flat.bitcast(mybir.dt.int32)
                nc.vector.tensor_scalar(flat_i, flat_i, 0x7FFF7FFF, None,
                                        mybir.AluOpType.bitwise_and)
            # dSE,dSW on ACT (before the drains of j-1 in the ACT queue so
            # the queue head never waits on the previous job's matmuls)
            nc.scalar.activation(dSE[:], dSE[:], Abs)
            nc.scalar.activation(dSW[:], dSW[:], Abs)

            # ---- pipelined late stages
            if pending is not None:
                drain_stage(pending[1], pending[2])
                fin_pending.append((pending[0], pending[2], pending[3]))
            if len(fin_pending) >= 2:
                final_stage(*fin_pending.pop(0))

            # ---- 8-term accumulate on PE: per bank (2 out rows) 4 pair
            # matmuls with diag(alpha) stationary
            o_t = o_pool.tile([128, J, W], BF16, name="o_t", tag="o")
            ps_list = []
            for r in range(J // 4):
                ps = ps_pool.tile([128, 4 * W], F32, name="ps", tag="ps")
                ps_list.append(ps)
                for g in range(2):      # one PSUM bank per g (2 rows x 256)
                    rr = 4 * r + 2 * g  # first pixel row (q-coord rr+1)
                    # W@ dE[rr..rr+1, 1:257]   E@ dE[rr..rr+1, 2:258]
                    # N@ dS[rr..rr+1, 0:256]   S@ dS[rr+1..rr+2, 0:256]
                    # NW@dSE[rr..rr+1, 0:256]  SE@dSE[rr+1..rr+2, 1:257]
                    # NE@dSW[rr..rr+1, 1:257]  SW@dSW[rr+1..rr+2, 0:256]
                    terms = (
                        (dE, rr * WD + 1, WD),
                        (dE, rr * WD + 2, WD),
                        (dS, rr * W, W),
                        (dS, (rr + 1) * W, W),
                        (dSE, rr * WD, WD),
                        (dSE, (rr + 1) * WD + 1, WD),
                        (dSW, rr * WD + 1, WD),
                        (dSW, (rr + 1) * WD, WD),
                    )
                    for t, (d_t, off, rstr) in enumerate(terms):
                        term_matmul(ps, g, d_t, off, rstr,
                                    start=(t == 0), stop=(t == len(terms) - 1))
            pending = (j, ps_list, o_t, xp)

        drain_stage(pending[1], pending[2])
        fin_pending.append((pending[0], pending[2], pending[3]))
        for fp in fin_pending:
            final_stage(*fp)

    nc.compile()
    return nc


def _prep_inputs(x, alpha, H=256, W=256):
    """Shard batch across cores, cast to bf16 and zero-pad on host."""
    import ml_dtypes
    x = np.asarray(x, dtype=np.float32)
    alpha = np.asarray(alpha, dtype=np.float32).reshape(C)
    B = x.shape[0]
    HP, WP = H + 2, W + 4
    adiag = np.zeros((128, 128), dtype=np.float32)
    idx = np.arange(128)
    adiag[idx, idx] = alpha[idx % C]
    adiag = adiag.astype(ml_dtypes.bfloat16)
    in_maps = []
    for i in range(B):
        xs = np.zeros((C, HP, WP), dtype=ml_dtypes.bfloat16)
        xs[:, 1:H + 1, 2:W + 2] = x[i].astype(ml_dtypes.bfloat16)
        in_maps.append({"x": xs, "adiag": adiag})
    return in_maps


_GRAPH_CACHE = {}


def _get_graph(H=256, W=256, J=16):
    key = (H, W, J)
    if key not in _GRAPH_CACHE:
        _GRAPH_CACHE[key] = build_graph(H, W, J)
    return _GRAPH_CACHE[key]


def kernel(x, alpha, _profile=False):
    x = np.asarray(x, dtype=np.float32)
    alpha = np.asarray(alpha, dtype=np.float32)
    B, c, H, W = x.shape
    assert c == C and B == N_CORES, (B, c, H, W)
    nc = _get_graph(H, W)
    in_maps = _prep_inputs(x, alpha, H, W)
    res = run_bass_kernel_spmd(nc, in_maps, core_ids=list(range(N_CORES)),
                               trace=_profile)
    out = np.stack([res.results[i]["out"].astype(np.float32)
                    for i in range(N_CORES)], axis=0)
    if _profile:
        return out, res
    return out


def kernel_profiled(x, alpha):
    out, res = kernel(x, alpha, _profile=True)
    return out, res.exec_time_ns


# revision 12
# speedup vs baseline: 1.1259x; 1.0321x over previous
"""Trainium2 Bass kernel for the 3x3 abs-diff stencil module:

    out = x + alpha * sum_{di,dj in 3x3} |x - shift_{di,dj}(zero_pad(x))|

x: (8, 64, 256, 256) f32, alpha: (1, 64, 1, 1) f32.

Strategy (pure data parallel, no collectives; core i <- batch i):

  - Host pre-casts x to bf16 and zero-pads each shard to (64, 258, 260).
  - SBUF layout: partition p = (s, c), s = H-half, c = channel; free dim =
    (rows, cols). All stencil shifts are free-dim AP offsets.
  - Per 16-row job, 4 signed diff fields are computed on DVE (bf16 2x):
      dE[t,u]  = xp[t+1,u]   - xp[t+1,u+1]   (horizontal)
      dS[t,w]  = xp[t,w+2]   - xp[t+1,w+2]   (vertical)
      dSE[t,u] = xp[t,u+1]   - xp[t+1,u+2]   (diagonal \\)
      dSW[t,u] = xp[t,u+2]   - xp[t+1,u+1]   (diagonal /)
    abs is split across engines: dE,dS via DVE int32-AND (2 int32/cyc),
    dSE,dSW via ACT Abs.
  - The 8-neighbor sum runs entirely on the PE: for each PSUM bank
    (2 output rows x 256 cols) 4 matmuls accumulate the 8 shifted terms,
    each matmul folding a +/- direction PAIR of the same field through a
    4-axis moving AP whose output AP repeats the bank slots (stride-0 pair
    axis -> PSUM accumulates both passes).  The stationary matrix is
    diag(alpha) in bf16 for every matmul in the kernel, so PSUM ends up
    holding alpha * S directly and the drain is a plain ACT copy to bf16.
  - DVE then adds x into the drained tile (out = x + alpha*S) and the
    result is stored as bf16 (host casts back to f32).
  - Emission is software-pipelined as in the v1 kernel: ACT abs of job j
    precede the PSUM drains of job j-1; final add + store of job j-2 are
    emitted inside iteration j.  Loads ride HWDGE on the idle SP queue,
    stores ride SWDGE on the otherwise idle Pool queue so neither blocks
    the other at a queue head.

Per-core engine budget (theory): DVE ~110us, PE ~109us, ACT ~93us.
"""

import sys

import numpy as np

try:
    import concourse  # noqa: F401
except ImportError:
    sys.path.insert(0, "/opt/trn_rl_repo")

from contextlib import ExitStack

import concourse.bacc as bacc
import concourse.bass as bass
import concourse.mybir as mybir
import concourse.tile as tile
from concourse.bass_utils import run_bass_kernel_spmd

F32 = mybir.dt.float32
BF16 = mybir.dt.bfloat16

C = 64
N_CORES = 8


def build_graph(H=256, W=256, J=16):
    """Build the per-core Bass graph (identical on all 8 cores).

    Input DRAM tensor per core: (C, H+2, W+4) bf16 host-padded;
    output (C, H, W) bf16; adiag (128, 128) bf16 = diag(alpha).
    """
    HP, WP = H + 2, W + 4
    HH = H // 2          # rows per half
    # Variable job sizes: small first jobs prime the pipeline (first load is
    # tiny, compute starts early), small last jobs shrink the drain tail.
    assert HH == 128 and J == 16
    jobs = [4, 12] + [16] * 6 + [12, 4]
    r0s = [sum(jobs[:i]) for i in range(len(jobs))]
    assert sum(jobs) == HH

    nc = bacc.Bacc("TRN2", target_bir_lowering=False, debug=False,
                   num_devices=N_CORES)
    x_d = nc.dram_tensor("x", [C, HP, WP], BF16, kind="ExternalInput")
    a_d = nc.dram_tensor("adiag", [128, 128], BF16, kind="ExternalInput")
    o_d = nc.dram_tensor("out", [C, H, W], BF16, kind="ExternalOutput")

    sub = mybir.AluOpType.subtract
    Copy = mybir.ActivationFunctionType.Copy
    Abs = mybir.ActivationFunctionType.Abs

    with tile.TileContext(nc) as tc, ExitStack() as ctx:
        const_pool = ctx.enter_context(tc.tile_pool(name="const", bufs=1))
        xp_pool = ctx.enter_context(tc.tile_pool(name="xp", bufs=4))
        d_pool = ctx.enter_context(tc.tile_pool(name="d", bufs=2))
        o_pool = ctx.enter_context(tc.tile_pool(name="o", bufs=3))
        ps_pool = ctx.enter_context(tc.tile_pool(name="ps", bufs=4, space="PSUM"))

        adiag_t = const_pool.tile([128, 128], BF16, name="adiag_t")
        nc.sync.dma_start(out=adiag_t[:], in_=a_d.ap())

        def drain_stage(ps_list, o_t):
            # ACT: PSUM (= alpha*S, f32) -> bf16 o_t rows; must complete
            # before the next job's matmuls reuse the banks
            for r, ps in enumerate(ps_list):
                nc.scalar.activation(o_t[:, 4 * r:4 * r + 4, :], ps[:], Copy)

        def final_stage(r0, Jj, o_t, xp):
            # DVE: out = alpha*S + x ; then SWDGE store on the Pool queue.
            # One flat free axis -> one contiguous descriptor per partition.
            nc.vector.tensor_add(o_t[:], o_t[:], xp[:, 1:Jj + 1, 2:W + 2])
            dst = bass.AP(o_d, r0 * W,
                          [[HH * W, 2], [H * W, C], [1, Jj * W]])
            nc.gpsimd.dma_start(out=dst,
                                in_=o_t[:].rearrange("p r w -> p (r w)"))

        def term_matmul(ps, g, d_t, elem_off, row_stride, start, stop):
            # one matmul accumulating one shifted |diff| term (2 rows x 256)
            # into PSUM bank slice g, stationary = diag(alpha)
            mv = bass.AP(d_t.tensor, d_t.offset + elem_off,
                         [list(d_t.ap[0]), [row_stride, 2], [1, W]])
            nc.tensor.matmul(ps[:, 512 * g:512 * g + 512], adiag_t[:], mv,
                             start=start, stop=stop)

        pending = None    # (r0, Jj, ps_list, o_t, xp) of the previous job
        for j, (r0, Jj) in enumerate(zip(r0s, jobs)):
            # ---- load (SWDGE; one contiguous descriptor per partition)
            xp = xp_pool.tile([128, Jj + 2, WP], BF16, name="xp", tag="xp")
            src = bass.AP(x_d, r0 * WP,
                          [[HH * WP, 2], [HP * WP, C], [1, (Jj + 2) * WP]])
            nc.gpsimd.dma_start(out=xp[:].rearrange("p r w -> p (r w)"),
                                in_=src)

            # ---- 4 signed diff fields on DVE (bf16 2x streams); the two
            # ACT-abs fields (dSE,dSW) first so ACT starts earliest
            dSE = d_pool.tile([128, Jj + 1, WP - 2], BF16, name="dSE", tag="dSE")
            dSW = d_pool.tile([128, Jj + 1, WP - 2], BF16, name="dSW", tag="dSW")
            dE = d_pool.tile([128, Jj, WP - 2], BF16, name="dE", tag="dE")
            dS = d_pool.tile([128, Jj + 1, W], BF16, name="dS", tag="dS")
            WD = WP - 2  # 258: diff-tile width

            nc.vector.tensor_tensor(dSE[:], xp[:, 0:Jj + 1, 1:WD + 1],
                                    xp[:, 1:Jj + 2, 2:WD + 2], sub)
            nc.scalar.activation(dSE[:], dSE[:], Abs)
            nc.vector.tensor_tensor(dSW[:], xp[:, 0:Jj + 1, 2:WD + 2],
                                    xp[:, 1:Jj + 2, 1:WD + 1], sub)
            nc.scalar.activation(dSW[:], dSW[:], Abs)
            nc.vector.tensor_tensor(dE[:], xp[:, 1:Jj + 1, 0:WD],
                                    xp[:, 1:Jj + 1, 1:WD + 1], sub)
            nc.vector.tensor_tensor(dS[:], xp[:, 0:Jj + 1, 2:W + 2],
                                    xp[:, 1:Jj + 2, 2:W + 2], sub)

            # ---- abs: dE,dS in place on DVE (int32 AND clears the packed
            # bf16 sign bits at 2 int32/cycle)
            for dt_ in (dE, dS):
                flat = dt_[:, :, :].rearrange("p r w -> p (r w)")
                flat_i = flat.bitcast(mybir.dt.int32)
                nc.vector.tensor_scalar(flat_i, flat_i, 0x7FFF7FFF, None,
                                        mybir.AluOpType.bitwise_and)

            # ---- pipelined late stages of job j-1: PSUM drains (ACT, after
            # this job's abs so the ACT queue head never waits on PE), then
            # final add + store
            if pending is not None:
                drain_stage(pending[2], pending[3])
                final_stage(pending[0], pending[1], pending[3], pending[4])

            # ---- 8-term accumulate on PE: per bank (2 out rows) 8 matmuls
            # with diag(alpha) stationary
            o_t = o_pool.tile([128, Jj, W], BF16, name="o_t", tag="o")
            ps_list = []
            for r in range(Jj // 4):
                ps = ps_pool.tile([128, 4 * W], F32, name="ps", tag="ps")
                ps_list.append(ps)
                for g in range(2):      # one PSUM bank per g (2 rows x 256)
                    rr = 4 * r + 2 * g  # first pixel row (q-coord rr+1)
                    # W@ dE[rr..rr+1, 1:257]   E@ dE[rr..rr+1, 2:258]
                    # N@ dS[rr..rr+1, 0:256]   S@ dS[rr+1..rr+2, 0:256]
                    # NW@dSE[rr..rr+1, 0:256]  SE@dSE[rr+1..rr+2, 1:257]
                    # NE@dSW[rr..rr+1, 1:257]  SW@dSW[rr+1..rr+2, 0:256]
                    terms = (
                        (dE, rr * WD + 1, WD),
                        (dE, rr * WD + 2, WD),
                        (dS, rr * W, W),
                        (dS, (rr + 1) * W, W),
                        (dSE, rr * WD, WD),
                        (dSE, (rr + 1) * WD + 1, WD),
                        (dSW, rr * WD + 1, WD),
                        (dSW, (rr + 1) * WD, WD),
                    )
                    for t, (d_t, off, rstr) in enumerate(terms):
                        term_matmul(ps, g, d_t, off, rstr,
                                    start=(t == 0), stop=(t == len(terms) - 1))
            pending = (r0, Jj, ps_list, o_t, xp)

        drain_stage(pending[2], pending[3])
        final_stage(pending[0], pending[1], pending[3], pending[4])

    nc.compile()
    return nc


def _prep_inputs(x, alpha, H=256, W=256):
    """Shard batch across cores, cast to bf16 and zero-pad on host."""
    import ml_dtypes
    x = np.asarray(x, dtype=np.float32)
    alpha = np.asarray(alpha, dtype=np.float32).reshape(C)
    B = x.shape[0]
    HP, WP = H + 2, W + 4
    adiag = np.zeros((128, 128), dtype=np.float32)
    idx = np.arange(128)
    adiag[idx, idx] = alpha[idx % C]
    adiag = adiag.astype(ml_dtypes.bfloat16)
    in_maps = []
    for i in range(B):
        xs = np.zeros((C, HP, WP), dtype=ml_dtypes.bfloat16)
        xs[:, 1:H + 1, 2:W + 2] = x[i].astype(ml_dtypes.bfloat16)
        in_maps.append({"x": xs, "adiag": adiag})
    return in_maps


_GRAPH_CACHE = {}


def _get_graph(H=256, W=256, J=16):
    key = (H, W, J)
    if key not in _GRAPH_CACHE:
        _GRAPH_CACHE[key] = build_graph(H, W, J)
    return _GRAPH_CACHE[key]


def kernel(x, alpha, _profile=False):
    x = np.asarray(x, dtype=np.float32)
    alpha = np.asarray(alpha, dtype=np.float32)
    B, c, H, W = x.shape
    assert c == C and B == N_CORES, (B, c, H, W)
    nc = _get_graph(H, W)
    in_maps = _prep_inputs(x, alpha, H, W)
    res = run_bass_kernel_spmd(nc, in_maps, core_ids=list(range(N_CORES)),
                               trace=_profile)
    out = np.stack([res.results[i]["out"].astype(np.float32)
                    for i in range(N_CORES)], axis=0)
    if _profile:
        return out, res
    return out


def kernel_profiled(x, alpha):
    out, res = kernel(x, alpha, _profile=True)
    return out, res.exec_time_ns


# revision 13
# speedup vs baseline: 1.1889x; 1.0560x over previous
"""Trainium2 Bass kernel for the 3x3 abs-diff stencil module:

    out = x + alpha * sum_{di,dj in 3x3} |x - shift_{di,dj}(zero_pad(x))|

x: (8, 64, 256, 256) f32, alpha: (1, 64, 1, 1) f32.

Strategy (pure data parallel, no collectives; core i <- batch i):

  - Host pre-casts x to bf16 and zero-pads each shard to (64, 258, 260).
  - SBUF layout: partition p = (s, c), s = H-half, c = channel; free dim =
    (rows, cols). All stencil shifts are free-dim AP offsets.
  - Per 16-row job, 4 signed diff fields are computed on DVE (bf16 2x):
      dE[t,u]  = xp[t+1,u]   - xp[t+1,u+1]   (horizontal)
      dS[t,w]  = xp[t,w+2]   - xp[t+1,w+2]   (vertical)
      dSE[t,u] = xp[t,u+1]   - xp[t+1,u+2]   (diagonal \\)
      dSW[t,u] = xp[t,u+2]   - xp[t+1,u+1]   (diagonal /)
    abs is split across engines: dE,dS via DVE int32-AND (2 int32/cyc),
    dSE,dSW via ACT Abs.
  - The 8-neighbor sum runs entirely on the PE: for each PSUM bank
    (2 output rows x 256 cols) 4 matmuls accumulate the 8 shifted terms,
    each matmul folding a +/- direction PAIR of the same field through a
    4-axis moving AP whose output AP repeats the bank slots (stride-0 pair
    axis -> PSUM accumulates both passes).  The stationary matrix is
    diag(alpha) in bf16 for every matmul in the kernel, so PSUM ends up
    holding alpha * S directly and the drain is a plain ACT copy to bf16.
  - DVE then adds x into the drained tile (out = x + alpha*S) and the
    result is stored as bf16 (host casts back to f32).
  - Emission is software-pipelined as in the v1 kernel: ACT abs of job j
    precede the PSUM drains of job j-1; final add + store of job j-2 are
    emitted inside iteration j.  Loads ride HWDGE on the idle SP queue,
    stores ride SWDGE on the otherwise idle Pool queue so neither blocks
    the other at a queue head.

Per-core engine budget (theory): DVE ~110us, PE ~109us, ACT ~93us.
"""

import sys

import numpy as np

try:
    import concourse  # noqa: F401
except ImportError:
    sys.path.insert(0, "/opt/trn_rl_repo")

from contextlib import ExitStack

import concourse.bacc as bacc
import concourse.bass as bass
import concourse.mybir as mybir
import concourse.tile as tile
from concourse.bass_utils import run_bass_kernel_spmd

F32 = mybir.dt.float32
BF16 = mybir.dt.bfloat16

C = 64
N_CORES = 8


def build_graph(H=256, W=256, J=16):
    """Build the per-core Bass graph (identical on all 8 cores).

    Input DRAM tensor per core: (C, H+2, W+4) bf16 host-padded;
    output (C, H, W) bf16; adiag (128, 128) bf16 = diag(alpha).
    """
    HP, WP = H + 2, W + 4
    HH = H // 2          # rows per half
    # Variable job sizes: small first jobs prime the pipeline (first load is
    # tiny, compute starts early), small last jobs shrink the drain tail.
    assert HH == 128 and J == 16
    jobs = [4, 12] + [16] * 6 + [12, 4]
    r0s = [sum(jobs[:i]) for i in range(len(jobs))]
    assert sum(jobs) == HH

    nc = bacc.Bacc("TRN2", target_bir_lowering=False, debug=False,
                   num_devices=N_CORES)
    x_d = nc.dram_tensor("x", [C, HP, WP], BF16, kind="ExternalInput")
    a_d = nc.dram_tensor("adiag", [128, 128], BF16, kind="ExternalInput")
    o_d = nc.dram_tensor("out", [C, H, W], BF16, kind="ExternalOutput")

    sub = mybir.AluOpType.subtract
    Copy = mybir.ActivationFunctionType.Copy
    Abs = mybir.ActivationFunctionType.Abs

    with tile.TileContext(nc) as tc, ExitStack() as ctx:
        const_pool = ctx.enter_context(tc.tile_pool(name="const", bufs=1))
        xp_pool = ctx.enter_context(tc.tile_pool(name="xp", bufs=4))
        d_pool = ctx.enter_context(tc.tile_pool(name="d", bufs=2))
        o_pool = ctx.enter_context(tc.tile_pool(name="o", bufs=3))
        ps_pool = ctx.enter_context(tc.tile_pool(name="ps", bufs=4, space="PSUM"))

        adiag_t = const_pool.tile([128, 128], BF16, name="adiag_t")
        nc.sync.dma_start(out=adiag_t[:], in_=a_d.ap())

        def drain_stage(ps_list, o_t):
            # ACT: PSUM (= alpha*S, f32) -> bf16 o_t rows; must complete
            # before the next job's matmuls reuse the banks
            for r, ps in enumerate(ps_list):
                nc.scalar.activation(o_t[:, 4 * r:4 * r + 4, :], ps[:], Copy)

        def final_stage(r0, Jj, o_t, xp):
            # DVE: out = alpha*S + x ; then HWDGE store on the SP queue so a
            # store gen waiting on its final-add never head-of-line blocks
            # the next load gen (loads own the Pool queue).
            nc.vector.tensor_add(o_t[:], o_t[:], xp[:, 1:Jj + 1, 2:W + 2])
            dst = bass.AP(o_d, r0 * W,
                          [[HH * W, 2], [H * W, C], [1, Jj * W]])
            nc.sync.dma_start(out=dst,
                              in_=o_t[:].rearrange("p r w -> p (r w)"))

        def term_matmul(ps, g, d_t, elem_off, row_stride, start, stop):
            # one matmul accumulating one shifted |diff| term (2 rows x 256)
            # into PSUM bank slice g, stationary = diag(alpha)
            mv = bass.AP(d_t.tensor, d_t.offset + elem_off,
                         [list(d_t.ap[0]), [row_stride, 2], [1, W]])
            nc.tensor.matmul(ps[:, 512 * g:512 * g + 512], adiag_t[:], mv,
                             start=start, stop=stop)

        pending = None    # (r0, Jj, ps_list, o_t, xp) of the previous job
        for j, (r0, Jj) in enumerate(zip(r0s, jobs)):
            # ---- load (SWDGE; one contiguous descriptor per partition)
            xp = xp_pool.tile([128, Jj + 2, WP], BF16, name="xp", tag="xp")
            src = bass.AP(x_d, r0 * WP,
                          [[HH * WP, 2], [HP * WP, C], [1, (Jj + 2) * WP]])
            nc.gpsimd.dma_start(out=xp[:].rearrange("p r w -> p (r w)"),
                                in_=src)

            # ---- 4 signed diff fields on DVE (bf16 2x streams); the two
            # ACT-abs fields (dSE,dSW) first so ACT starts earliest
            dSE = d_pool.tile([128, Jj + 1, WP - 2], BF16, name="dSE", tag="dSE")
            dSW = d_pool.tile([128, Jj + 1, WP - 2], BF16, name="dSW", tag="dSW")
            dE = d_pool.tile([128, Jj, WP - 2], BF16, name="dE", tag="dE")
            dS = d_pool.tile([128, Jj + 1, W], BF16, name="dS", tag="dS")
            WD = WP - 2  # 258: diff-tile width

            nc.vector.tensor_tensor(dSE[:], xp[:, 0:Jj + 1, 1:WD + 1],
                                    xp[:, 1:Jj + 2, 2:WD + 2], sub)
            nc.scalar.activation(dSE[:], dSE[:], Abs)
            nc.vector.tensor_tensor(dSW[:], xp[:, 0:Jj + 1, 2:WD + 2],
                                    xp[:, 1:Jj + 2, 1:WD + 1], sub)
            nc.scalar.activation(dSW[:], dSW[:], Abs)
            nc.vector.tensor_tensor(dE[:], xp[:, 1:Jj + 1, 0:WD],
                                    xp[:, 1:Jj + 1, 1:WD + 1], sub)
            nc.vector.tensor_tensor(dS[:], xp[:, 0:Jj + 1, 2:W + 2],
                                    xp[:, 1:Jj + 2, 2:W + 2], sub)

            # ---- abs: dE,dS in place on DVE (int32 AND clears the packed
            # bf16 sign bits at 2 int32/cycle)
            for dt_ in (dE, dS):
                flat = dt_[:, :, :].rearrange("p r w -> p (r w)")
                flat_i = flat.bitcast(mybir.dt.int32)
                nc.vector.tensor_scalar(flat_i, flat_i, 0x7FFF7FFF, None,
                                        mybir.AluOpType.bitwise_and)

            # ---- pipelined late stages of job j-1: PSUM drains (ACT, after
            # this job's abs so the ACT queue head never waits on PE), then
            # final add + store
            if pending is not None:
                drain_stage(pending[2], pending[3])
                final_stage(pending[0], pending[1], pending[3], pending[4])

            # ---- 8-term accumulate on PE: per bank (2 out rows) 8 matmuls
            # with diag(alpha) stationary
            o_t = o_pool.tile([128, Jj, W], BF16, name="o_t", tag="o")
            ps_list = []
            for r in range(Jj // 4):
                ps = ps_pool.tile([128, 4 * W], F32, name="ps", tag="ps")
                ps_list.append(ps)
                for g in range(2):      # one PSUM bank per g (2 rows x 256)
                    rr = 4 * r + 2 * g  # first pixel row (q-coord rr+1)
                    # W@ dE[rr..rr+1, 1:257]   E@ dE[rr..rr+1, 2:258]
                    # N@ dS[rr..rr+1, 0:256]   S@ dS[rr+1..rr+2, 0:256]
                    # NW@dSE[rr..rr+1, 0:256]  SE@dSE[rr+1..rr+2, 1:257]
                    # NE@dSW[rr..rr+1, 1:257]  SW@dSW[rr+1..rr+2, 0:256]
                    terms = (
                        (dE, rr * WD + 1, WD),
                        (dE, rr * WD + 2, WD),
                        (dS, rr * W, W),
                        (dS, (rr + 1) * W, W),
                        (dSE, rr * WD, WD),
                        (dSE, (rr + 1) * WD + 1, WD),
                        (dSW, rr * WD + 1, WD),
                        (dSW, (rr + 1) * WD, WD),
                    )
                    for t, (d_t, off, rstr) in enumerate(terms):
                        term_matmul(ps, g, d_t, off, rstr,
                                    start=(t == 0), stop=(t == len(terms) - 1))
            pending = (r0, Jj, ps_list, o_t, xp)

        drain_stage(pending[2], pending[3])
        final_stage(pending[0], pending[1], pending[3], pending[4])

    nc.compile()
    return nc


def _prep_inputs(x, alpha, H=256, W=256):
    """Shard batch across cores, cast to bf16 and zero-pad on host."""
    import ml_dtypes
    x = np.asarray(x, dtype=np.float32)
    alpha = np.asarray(alpha, dtype=np.float32).reshape(C)
    B = x.shape[0]
    HP, WP = H + 2, W + 4
    adiag = np.zeros((128, 128), dtype=np.float32)
    idx = np.arange(128)
    adiag[idx, idx] = alpha[idx % C]
    adiag = adiag.astype(ml_dtypes.bfloat16)
    in_maps = []
    for i in range(B):
        xs = np.zeros((C, HP, WP), dtype=ml_dtypes.bfloat16)
        xs[:, 1:H + 1, 2:W + 2] = x[i].astype(ml_dtypes.bfloat16)
        in_maps.append({"x": xs, "adiag": adiag})
    return in_maps


_GRAPH_CACHE = {}


def _get_graph(H=256, W=256, J=16):
    key = (H, W, J)
    if key not in _GRAPH_CACHE:
        _GRAPH_CACHE[key] = build_graph(H, W, J)
    return _GRAPH_CACHE[key]


def kernel(x, alpha, _profile=False):
    x = np.asarray(x, dtype=np.float32)
    alpha = np.asarray(alpha, dtype=np.float32)
    B, c, H, W = x.shape
    assert c == C and B == N_CORES, (B, c, H, W)
    nc = _get_graph(H, W)
    in_maps = _prep_inputs(x, alpha, H, W)
    res = run_bass_kernel_spmd(nc, in_maps, core_ids=list(range(N_CORES)),
                               trace=_profile)
    out = np.stack([res.results[i]["out"].astype(np.float32)
                    for i in range(N_CORES)], axis=0)
    if _profile:
        return out, res
    return out


def kernel_profiled(x, alpha):
    out, res = kernel(x, alpha, _profile=True)
    return out, res.exec_time_ns
